# revision 4
# baseline (speedup 1.0000x reference)
"""Multi-head attention (B=4, S=2048, D=1024, H=16) on 8 trn2 NeuronCores.

Sharding: (batch, head-half) -> one core each. Core c handles batch c//2 and
heads (c%2)*8 .. (c%2)*8+7 (feature columns (c%2)*512 .. +512 of the QKV
projections, rows of Wo). Each core computes its 8 heads' attention and a
partial output projection; the host sums the two partials per batch and adds
the output bias.

Device layout per core (S=2048 tokens, F=512 local features, hd=64):
  - inputs Q/K/V arrive host-transposed as [1024, 2048] so the d_model
    contraction sits on SBUF partitions,
  - q^T/k^T are produced feature-major ([512, 2048]) via lhsT=W, rhs=X^T,
  - scores are computed transposed (S^T[k, q]) so the P@V matmul can use v
    in natural [token, feature] layout as the stationary operand,
  - softmax: exp on the ACT engine with the 1/8 scale folded in; the
    denominator comes from an all-ones 65th column appended to v; the
    normalization multiplies o'^T rows by a PE-broadcast reciprocal row.
Matmuls run as float32r (FP22 multiplies, fp32 accumulate) except P@V,
which runs bf16 x bf16 on the exp output.
"""
import numpy as np

import concourse.bass as bass
import concourse.tile as tile
from concourse import mybir
from concourse.bass_utils import run_bass_kernel_spmd

F32 = mybir.dt.float32
F32R = mybir.dt.float32r
BF16 = mybir.dt.bfloat16
EXP = mybir.ActivationFunctionType.Exp

B, S, DM, H_TOT = 4, 2048, 1024, 16
F = 512          # features per core (8 heads x 64)
HD = 64          # head dim
NH = 8           # heads per core
NP = 4           # head pairs per core
KT = 16          # k tiles of 128
NQT = 4          # q chunks of 512
SCALE = 0.125    # 1/sqrt(64)
N_CORES = 8

_WAIT_CAP = {"InstEventSemaphore": 2}


def _split_multiwaits(nc):
    """This walrus build accepts 1 sync-wait per instruction (2 on
    EventSemaphore); spread extras over same-engine NOPs placed before."""
    n_spill = 0
    for f in nc.m.functions:
        for bb in f.blocks:
            new = []
            changed = False
            for inst in bb.instructions:
                si = inst.sync_info
                cap = _WAIT_CAP.get(type(inst).__name__, 1)
                if si is not None and len(si.on_wait) > cap:
                    extra = list(si.on_wait[: len(si.on_wait) - cap])
                    del si.on_wait[: len(si.on_wait) - cap]
                    for w in extra:
                        n_spill += 1
                        nop = mybir.InstNoOp(name=f"I-wspill-{n_spill}-{inst.name}")
                        nop.engine = inst.engine
                        nop.sync_info = mybir.SyncInfo(on_wait=[w], on_update=[])
                        new.append(nop)
                    changed = True
                new.append(inst)
            if changed:
                bb.instructions[:] = new
    return n_spill


def build_program():
    nc = bass.Bass("TRN2", target_bir_lowering=False, debug=False, num_devices=1)

    d_qt = nc.dram_tensor("qt", [DM, S], F32R, kind="ExternalInput").ap()
    d_kt = nc.dram_tensor("kt", [DM, S], F32R, kind="ExternalInput").ap()
    d_vt = nc.dram_tensor("vt", [DM, S], F32R, kind="ExternalInput").ap()
    d_wq = nc.dram_tensor("wq", [DM, F], F32R, kind="ExternalInput").ap()
    d_wk = nc.dram_tensor("wk", [DM, F], F32R, kind="ExternalInput").ap()
    d_wv = nc.dram_tensor("wv", [DM, F], F32R, kind="ExternalInput").ap()
    d_wo = nc.dram_tensor("wo", [F, DM], F32R, kind="ExternalInput").ap()
    d_bq = nc.dram_tensor("bq", [F], F32, kind="ExternalInput").ap()
    d_bk = nc.dram_tensor("bk", [F], F32, kind="ExternalInput").ap()
    d_bv = nc.dram_tensor("bv", [F], F32R, kind="ExternalInput").ap()
    d_ones = nc.dram_tensor("ones", [1, 128], F32R, kind="ExternalInput").ap()
    d_part = nc.dram_tensor("part", [S, DM], F32, kind="ExternalOutput").ap()

    with tile.TileContext(nc) as tc:
        with (
            tc.tile_pool(name="wpool", bufs=1) as wpool,
            tc.tile_pool(name="big", bufs=1) as big,
            tc.tile_pool(name="inch", bufs=4) as inch,
            tc.tile_pool(name="wch", bufs=4) as wch,
            tc.tile_pool(name="vtch", bufs=4) as vtch,
            tc.tile_pool(name="exch", bufs=4) as exch,
            tc.tile_pool(name="small", bufs=4) as small,
            tc.tile_pool(name="outst", bufs=2) as outst,
            tc.tile_pool(name="rcp", bufs=2) as rcp,
            tc.tile_pool(name="ps_sc", bufs=2, space="PSUM") as ps_sc,
            tc.tile_pool(name="ps_pv", bufs=2, space="PSUM") as ps_pv,
            tc.tile_pool(name="ps_misc", bufs=2, space="PSUM") as ps_misc,
        ):
            # ---- resident tiles
            wv_sb = [wpool.tile([128, F], F32R, tag=f"wv{m}", name=f"wv{m}") for m in range(8)]
            wo_sb = [wpool.tile([128, DM], F32R, tag=f"wo{f}", name=f"wo{f}") for f in range(4)]
            qT_sb = [big.tile([128, S], F32R, tag=f"qT{f}", name=f"qT{f}") for f in range(4)]
            kT_sb = [big.tile([128, S], F32R, tag=f"kT{f}", name=f"kT{f}") for f in range(4)]
            oT_sb = [big.tile([128, S], F32R, tag=f"oT{f}", name=f"oT{f}") for f in range(4)]
            v_sb = [big.tile([128, NH * (HD + 1)], BF16, tag=f"v{t}", name=f"v{t}") for t in range(KT)]
            bq_sb = wpool.tile([128, 4], F32, tag="bq")
            bk_sb = wpool.tile([128, 4], F32, tag="bk")
            bv_sb = wpool.tile([1, F], F32R, tag="bv")
            ones_sb = wpool.tile([1, 128], F32R, tag="ones")
            bvbc_sb = wpool.tile([128, F], F32, tag="bvbc")

            for m in range(8):
                nc.sync.dma_start(wv_sb[m][:], d_wv[128 * m:128 * (m + 1), :])
            for f in range(4):
                nc.sync.dma_start(wo_sb[f][:], d_wo[128 * f:128 * (f + 1), :])
            nc.sync.dma_start(bq_sb[:], d_bq.rearrange("(f p) -> p f", p=128))
            nc.sync.dma_start(bk_sb[:], d_bk.rearrange("(f p) -> p f", p=128))
            nc.sync.dma_start(bv_sb[:], d_bv.rearrange("(a f) -> a f", a=1))
            nc.sync.dma_start(ones_sb[:], d_ones[:])

            # bv broadcast over partitions via K=1 matmul (biases are usually
            # zero here, but keep the math general)
            psbv = ps_misc.tile([128, 512], F32, tag="ps")
            nc.tensor.matmul(psbv[:], ones_sb[0:1, :], bv_sb[0:1, :])
            nc.vector.tensor_copy(bvbc_sb[:], psbv[:])

            # ---- projections producing transposed outputs: k^T then q^T
            def project_T(w_dram, src, bias_sb, dst_sb):
                for n in range(NQT):
                    for fh in range(2):
                        acc = [
                            ps_misc.tile([128, 512], F32, tag="ps", name="acc")
                            for _ in range(2)
                        ]
                        for m in range(8):
                            wc = wch.tile([128, 256], F32R, tag="wch", name="wc")
                            nc.sync.dma_start(
                                wc[:],
                                w_dram[128 * m:128 * (m + 1),
                                       256 * fh:256 * (fh + 1)],
                            )
                            ch = inch.tile([128, 512], F32R, tag="inch")
                            nc.sync.dma_start(
                                ch[:],
                                src[128 * m:128 * (m + 1), 512 * n:512 * (n + 1)],
                            )
                            for i in range(2):
                                nc.tensor.matmul(
                                    acc[i][:],
                                    wc[:, 128 * i:128 * (i + 1)],
                                    ch[:],
                                    start=(m == 0),
                                    stop=(m == 7),
                                )
                        for i in range(2):
                            f = 2 * fh + i
                            nc.vector.tensor_scalar_add(
                                dst_sb[f][:, 512 * n:512 * (n + 1)],
                                acc[i][:],
                                bias_sb[:, f:f + 1],
                            )

            project_T(d_wk, d_kt, bk_sb, kT_sb)

            # ---- v projection (natural layout, bf16, ones column per head)
            for t in range(KT):
                acc = ps_misc.tile([128, 512], F32, tag="ps")
                for m in range(8):
                    ch = vtch.tile([128, 128], F32R, tag="vtch")
                    nc.sync.dma_start(
                        ch[:], d_vt[128 * m:128 * (m + 1), 128 * t:128 * (t + 1)]
                    )
                    nc.tensor.matmul(
                        acc[:], ch[:], wv_sb[m][:], start=(m == 0), stop=(m == 7)
                    )
                v3 = v_sb[t][:].rearrange("p (h e) -> p h e", e=HD + 1)
                nc.vector.memset(v3[:, :, HD:HD + 1], 1.0)
                nc.vector.tensor_add(
                    v3[:, :, 0:HD],
                    acc[:].rearrange("p (h e) -> p h e", e=HD),
                    bvbc_sb[:].rearrange("p (h e) -> p h e", e=HD),
                )

            project_T(d_wq, d_qt, bq_sb, qT_sb)

            # ---- attention + output projection, q-chunk major
            for n in range(NQT):
                qsl = slice(512 * n, 512 * (n + 1))
                for p in range(NP):
                    poA = ps_pv.tile([128, 512], F32, tag="po")
                    poB = ps_pv.tile([128, 512], F32, tag="po")

                    def sc_emit(m, p=p, qsl=qsl):
                        scp = ps_sc.tile([128, 1024], F32, tag="sc")
                        ksl = slice(128 * m, 128 * (m + 1))
                        nc.tensor.matmul(
                            scp[:, 0:512], kT_sb[p][0:64, ksl], qT_sb[p][0:64, qsl],
                            tile_position=(0, 0),
                        )
                        nc.tensor.matmul(
                            scp[:, 512:1024], kT_sb[p][64:128, ksl],
                            qT_sb[p][64:128, qsl], tile_position=(64, 0),
                        )
                        ex = exch.tile([128, 1024], BF16, tag="ex")
                        nc.scalar.activation(ex[:], scp[:], EXP, scale=SCALE)
                        return ex

                    exs = {0: sc_emit(0), 1: sc_emit(1)}
                    for m in range(KT):
                        if m + 2 < KT:
                            exs[m + 2] = sc_emit(m + 2)
                        ex = exs.pop(m)
                        nc.tensor.matmul(
                            poA[0:65, :], v_sb[m][:, 130 * p:130 * p + 65],
                            ex[:, 0:512], start=(m == 0), stop=(m == KT - 1),
                        )
                        nc.tensor.matmul(
                            poB[0:65, :], v_sb[m][:, 130 * p + 65:130 * p + 130],
                            ex[:, 512:1024], start=(m == 0), stop=(m == KT - 1),
                        )
                    for i, po in ((0, poA), (1, poB)):
                        r0 = 64 * i
                        dn = small.tile([1, 512], F32R, tag="dn", name="dn")
                        nc.vector.tensor_copy(dn[0:1, :], po[64:65, :])
                        dnr = small.tile([1, 512], F32R, tag="dn", name="dnr")
                        with nc.allow_low_precision(reason="f32r reciprocal"):
                            nc.vector.reciprocal(dnr[0:1, :], dn[0:1, :])
                        pb = ps_misc.tile([128, 512], F32, tag="ps")
                        nc.tensor.matmul(pb[0:64, :], ones_sb[0:1, 0:64], dnr[0:1, :])
                        rc = rcp.tile([64, 512], F32, tag="rc", name="rc")
                        nc.vector.tensor_copy(rc[:], pb[0:64, :])
                        with nc.allow_low_precision(reason="f32r normalized out"):
                            nc.vector.tensor_mul(
                                oT_sb[p][r0:r0 + 64, qsl], po[0:64, :], rc[:]
                            )
                # Wo for this token chunk
                for t in range(4):
                    tt = 4 * n + t
                    tsl = slice(128 * tt, 128 * (tt + 1))
                    for j in range(2):
                        pw = ps_misc.tile([128, 512], F32, tag="ps")
                        for f in range(4):
                            nc.tensor.matmul(
                                pw[:], oT_sb[f][:, tsl],
                                wo_sb[f][:, 512 * j:512 * (j + 1)],
                                start=(f == 0), stop=(f == 3),
                            )
                        ost = outst.tile([128, 512], F32, tag="outst")
                        nc.vector.tensor_copy(ost[:], pw[:])
                        nc.sync.dma_start(
                            d_part[tsl, 512 * j:512 * (j + 1)], ost[:]
                        )

    _split_multiwaits(nc)
    return nc


_PROGRAM = None


def _get_program():
    global _PROGRAM
    if _PROGRAM is None:
        _PROGRAM = build_program()
    return _PROGRAM


def make_in_maps(Q, K, V, Wq, bq, Wk, bk, Wv, bv, Wo, bo):
    f32 = lambda x: np.asarray(x, dtype=np.float32)
    Q, K, V = f32(Q), f32(K), f32(V)
    Wq, Wk, Wv, Wo = f32(Wq), f32(Wk), f32(Wv), f32(Wo)
    bq, bk, bv = f32(bq), f32(bk), f32(bv)
    ones = np.ones((1, 128), np.float32)
    in_maps = []
    for c in range(N_CORES):
        b, hh = c // 2, c % 2
        fs = slice(F * hh, F * (hh + 1))
        in_maps.append({
            "qt": np.ascontiguousarray(Q[b].T),
            "kt": np.ascontiguousarray(K[b].T),
            "vt": np.ascontiguousarray(V[b].T),
            "wq": np.ascontiguousarray(Wq[:, fs]),
            "wk": np.ascontiguousarray(Wk[:, fs]),
            "wv": np.ascontiguousarray(Wv[:, fs]),
            "wo": np.ascontiguousarray(Wo[fs, :]),
            "bq": np.ascontiguousarray(bq[fs]),
            "bk": np.ascontiguousarray(bk[fs]),
            "bv": np.ascontiguousarray(bv[fs]),
            "ones": ones,
        })
    return in_maps


def kernel(Q, K, V, Wq, bq, Wk, bk, Wv, bv, Wo, bo, _trace=False, _trace_kwargs=None):
    nc = _get_program()
    in_maps = make_in_maps(Q, K, V, Wq, bq, Wk, bk, Wv, bv, Wo, bo)
    res = run_bass_kernel_spmd(
        nc, in_maps, core_ids=list(range(N_CORES)),
        trace=_trace, **(_trace_kwargs or {}),
    )
    parts = [r["part"] for r in res.results]
    out = np.stack([parts[2 * b] + parts[2 * b + 1] for b in range(B)])
    out += np.asarray(bo, dtype=np.float32)[None, None, :]
    if _trace:
        return out, res
    return out


# revision 5
# speedup vs baseline: 1.0436x; 1.0436x over previous
"""Multi-head attention (B=4, S=2048, D=1024, H=16) on 8 trn2 NeuronCores.

Sharding: (batch, head-half) -> one core each. Core c handles batch c//2 and
heads (c%2)*8 .. (c%2)*8+7 (feature columns (c%2)*512 .. +512 of the QKV
projections, rows of Wo). Each core computes its 8 heads' attention and a
partial output projection; the host sums the two partials per batch and adds
the output bias.

Device layout per core (S=2048 tokens, F=512 local features, hd=64):
  - inputs Q/K/V arrive host-transposed as [1024, 2048] so the d_model
    contraction sits on SBUF partitions,
  - q^T/k^T are produced feature-major ([512, 2048]) via lhsT=W, rhs=X^T,
  - scores are computed transposed (S^T[k, q]) so the P@V matmul can use v
    in natural [token, feature] layout as the stationary operand,
  - softmax: exp on the ACT engine with the 1/8 scale folded in; the
    denominator comes from an all-ones 65th column appended to v; the
    normalization multiplies o'^T rows by a PE-broadcast reciprocal row.
Matmuls run as float32r (FP22 multiplies, fp32 accumulate) except P@V,
which runs bf16 x bf16 on the exp output.
"""
import numpy as np

import concourse.bass as bass
import concourse.tile as tile
from concourse import mybir
from concourse.bass_utils import run_bass_kernel_spmd

F32 = mybir.dt.float32
F32R = mybir.dt.float32r
BF16 = mybir.dt.bfloat16
EXP = mybir.ActivationFunctionType.Exp

B, S, DM, H_TOT = 4, 2048, 1024, 16
F = 512          # features per core (8 heads x 64)
HD = 64          # head dim
NH = 8           # heads per core
NP = 4           # head pairs per core
KT = 16          # k tiles of 128
NQT = 4          # q chunks of 512
SCALE = 0.125    # 1/sqrt(64)
N_CORES = 8

_WAIT_CAP = {"InstEventSemaphore": 2}


def _split_multiwaits(nc):
    """This walrus build accepts 1 sync-wait per instruction (2 on
    EventSemaphore); spread extras over same-engine NOPs placed before."""
    n_spill = 0
    for f in nc.m.functions:
        for bb in f.blocks:
            new = []
            changed = False
            for inst in bb.instructions:
                si = inst.sync_info
                cap = _WAIT_CAP.get(type(inst).__name__, 1)
                if si is not None and len(si.on_wait) > cap:
                    extra = list(si.on_wait[: len(si.on_wait) - cap])
                    del si.on_wait[: len(si.on_wait) - cap]
                    for w in extra:
                        n_spill += 1
                        nop = mybir.InstNoOp(name=f"I-wspill-{n_spill}-{inst.name}")
                        nop.engine = inst.engine
                        nop.sync_info = mybir.SyncInfo(on_wait=[w], on_update=[])
                        new.append(nop)
                    changed = True
                new.append(inst)
            if changed:
                bb.instructions[:] = new
    return n_spill


def build_program():
    nc = bass.Bass("TRN2", target_bir_lowering=False, debug=False, num_devices=1)

    d_qt = nc.dram_tensor("qt", [DM, S], F32R, kind="ExternalInput").ap()
    d_kt = nc.dram_tensor("kt", [DM, S], F32R, kind="ExternalInput").ap()
    d_vt = nc.dram_tensor("vt", [DM, S], F32R, kind="ExternalInput").ap()
    d_wq = nc.dram_tensor("wq", [DM, F], F32R, kind="ExternalInput").ap()
    d_wk = nc.dram_tensor("wk", [DM, F], F32R, kind="ExternalInput").ap()
    d_wv = nc.dram_tensor("wv", [DM, F], F32R, kind="ExternalInput").ap()
    d_wo = nc.dram_tensor("wo", [F, DM], F32R, kind="ExternalInput").ap()
    d_bq = nc.dram_tensor("bq", [F], F32, kind="ExternalInput").ap()
    d_bk = nc.dram_tensor("bk", [F], F32, kind="ExternalInput").ap()
    d_bv = nc.dram_tensor("bv", [F], F32R, kind="ExternalInput").ap()
    d_ones = nc.dram_tensor("ones", [1, 128], F32R, kind="ExternalInput").ap()
    d_part = nc.dram_tensor("part", [S, DM], F32, kind="ExternalOutput").ap()

    with tile.TileContext(nc) as tc:
        with (
            tc.tile_pool(name="wpool", bufs=1) as wpool,
            tc.tile_pool(name="big", bufs=1) as big,
            tc.tile_pool(name="inch", bufs=4) as inch,
            tc.tile_pool(name="wch", bufs=4) as wch,
            tc.tile_pool(name="vtch", bufs=4) as vtch,
            tc.tile_pool(name="exch", bufs=6) as exch,
            tc.tile_pool(name="small", bufs=4) as small,
            tc.tile_pool(name="outst", bufs=2) as outst,
            tc.tile_pool(name="rcp", bufs=2) as rcp,
            tc.tile_pool(name="ps_sc", bufs=2, space="PSUM") as ps_sc,
            tc.tile_pool(name="ps_pv", bufs=3, space="PSUM") as ps_pv,
            tc.tile_pool(name="ps_misc", bufs=1, space="PSUM") as ps_misc,
        ):
            # ---- resident tiles
            wv_sb = [wpool.tile([128, F], F32R, tag=f"wv{m}", name=f"wv{m}") for m in range(8)]
            wo_sb = [wpool.tile([128, DM], F32R, tag=f"wo{f}", name=f"wo{f}") for f in range(4)]
            qT_sb = [big.tile([128, S], BF16, tag=f"qT{f}", name=f"qT{f}") for f in range(4)]
            kT_sb = [big.tile([128, S], BF16, tag=f"kT{f}", name=f"kT{f}") for f in range(4)]
            oT_sb = [big.tile([128, S], F32R, tag=f"oT{f}", name=f"oT{f}") for f in range(4)]
            v_sb = [big.tile([128, NH * (HD + 1)], BF16, tag=f"v{t}", name=f"v{t}") for t in range(KT)]
            bq_sb = wpool.tile([128, 4], F32, tag="bq")
            bk_sb = wpool.tile([128, 4], F32, tag="bk")
            bv_sb = wpool.tile([1, F], F32R, tag="bv")
            ones_sb = wpool.tile([1, 128], F32R, tag="ones")
            bvbc_sb = wpool.tile([128, F], F32, tag="bvbc")

            for m in range(8):
                nc.sync.dma_start(wv_sb[m][:], d_wv[128 * m:128 * (m + 1), :])
            for f in range(4):
                nc.sync.dma_start(wo_sb[f][:], d_wo[128 * f:128 * (f + 1), :])
            nc.sync.dma_start(bq_sb[:], d_bq.rearrange("(f p) -> p f", p=128))
            nc.sync.dma_start(bk_sb[:], d_bk.rearrange("(f p) -> p f", p=128))
            nc.sync.dma_start(bv_sb[:], d_bv.rearrange("(a f) -> a f", a=1))
            nc.sync.dma_start(ones_sb[:], d_ones[:])

            # bv broadcast over partitions via K=1 matmul (biases are usually
            # zero here, but keep the math general)
            psbv = ps_misc.tile([128, 512], F32, tag="ps")
            nc.tensor.matmul(psbv[:], ones_sb[0:1, :], bv_sb[0:1, :])
            nc.vector.tensor_copy(bvbc_sb[:], psbv[:])

            # ---- projections producing transposed outputs: k^T then q^T
            def project_T(w_dram, src, bias_sb, dst_sb):
                for n in range(NQT):
                    for fh in range(2):
                        acc = [
                            ps_misc.tile([128, 512], F32, tag="ps", name="acc0"),
                            ps_pv.tile([128, 512], F32, tag="po", name="acc1"),
                        ]
                        for m in range(8):
                            wc = wch.tile([128, 256], F32R, tag="wch", name="wc")
                            nc.sync.dma_start(
                                wc[:],
                                w_dram[128 * m:128 * (m + 1),
                                       256 * fh:256 * (fh + 1)],
                            )
                            ch = inch.tile([128, 512], F32R, tag="inch")
                            nc.sync.dma_start(
                                ch[:],
                                src[128 * m:128 * (m + 1), 512 * n:512 * (n + 1)],
                            )
                            for i in range(2):
                                nc.tensor.matmul(
                                    acc[i][:],
                                    wc[:, 128 * i:128 * (i + 1)],
                                    ch[:],
                                    start=(m == 0),
                                    stop=(m == 7),
                                )
                        for i in range(2):
                            f = 2 * fh + i
                            with nc.allow_low_precision(reason="bf16 qT/kT store"):
                                nc.vector.tensor_scalar_add(
                                    dst_sb[f][:, 512 * n:512 * (n + 1)],
                                    acc[i][:],
                                    bias_sb[:, f:f + 1],
                                )

            project_T(d_wk, d_kt, bk_sb, kT_sb)

            # ---- v projection (natural layout, bf16, ones column per head)
            for t in range(KT):
                acc = ps_misc.tile([128, 512], F32, tag="ps")
                for m in range(8):
                    ch = vtch.tile([128, 128], F32R, tag="vtch")
                    nc.sync.dma_start(
                        ch[:], d_vt[128 * m:128 * (m + 1), 128 * t:128 * (t + 1)]
                    )
                    nc.tensor.matmul(
                        acc[:], ch[:], wv_sb[m][:], start=(m == 0), stop=(m == 7)
                    )
                v3 = v_sb[t][:].rearrange("p (h e) -> p h e", e=HD + 1)
                nc.vector.memset(v3[:, :, HD:HD + 1], 1.0)
                nc.vector.tensor_add(
                    v3[:, :, 0:HD],
                    acc[:].rearrange("p (h e) -> p h e", e=HD),
                    bvbc_sb[:].rearrange("p (h e) -> p h e", e=HD),
                )

            project_T(d_wq, d_qt, bq_sb, qT_sb)

            # ---- attention + output projection, q-chunk major
            for n in range(NQT):
                qsl = slice(512 * n, 512 * (n + 1))
                for p in range(NP):
                    poA = ps_pv.tile([128, 512], F32, tag="po")
                    poB = ps_pv.tile([128, 512], F32, tag="po")

                    def sc_emit(m, p=p, qsl=qsl):
                        scp = ps_sc.tile([128, 1024], F32, tag="sc")
                        ksl = slice(128 * m, 128 * (m + 1))
                        nc.tensor.matmul(
                            scp[:, 0:512], kT_sb[p][0:64, ksl], qT_sb[p][0:64, qsl],
                            tile_position=(0, 0),
                        )
                        nc.tensor.matmul(
                            scp[:, 512:1024], kT_sb[p][64:128, ksl],
                            qT_sb[p][64:128, qsl], tile_position=(64, 0),
                        )
                        ex = exch.tile([128, 1024], BF16, tag="ex")
                        nc.scalar.activation(ex[:], scp[:], EXP, scale=SCALE)
                        return ex

                    exs = {0: sc_emit(0), 1: sc_emit(1)}
                    for m in range(KT):
                        if m + 2 < KT:
                            exs[m + 2] = sc_emit(m + 2)
                        ex = exs.pop(m)
                        nc.tensor.matmul(
                            poA[0:65, :], v_sb[m][:, 130 * p:130 * p + 65],
                            ex[:, 0:512], start=(m == 0), stop=(m == KT - 1),
                        )
                        nc.tensor.matmul(
                            poB[0:65, :], v_sb[m][:, 130 * p + 65:130 * p + 130],
                            ex[:, 512:1024], start=(m == 0), stop=(m == KT - 1),
                        )
                    for i, po in ((0, poA), (1, poB)):
                        r0 = 64 * i
                        dn = small.tile([1, 512], F32R, tag="dn", name="dn")
                        nc.vector.tensor_copy(dn[0:1, :], po[64:65, :])
                        dnr = small.tile([1, 512], F32R, tag="dn", name="dnr")
                        with nc.allow_low_precision(reason="f32r reciprocal"):
                            nc.vector.reciprocal(dnr[0:1, :], dn[0:1, :])
                        pb = ps_misc.tile([128, 512], F32, tag="ps")
                        nc.tensor.matmul(pb[0:64, :], ones_sb[0:1, 0:64], dnr[0:1, :])
                        rc = rcp.tile([64, 512], F32, tag="rc", name="rc")
                        nc.vector.tensor_copy(rc[:], pb[0:64, :])
                        with nc.allow_low_precision(reason="f32r normalized out"):
                            nc.vector.tensor_mul(
                                oT_sb[p][r0:r0 + 64, qsl], po[0:64, :], rc[:]
                            )
                # Wo for this token chunk
                for t in range(4):
                    tt = 4 * n + t
                    tsl = slice(128 * tt, 128 * (tt + 1))
                    for j in range(2):
                        pw = ps_misc.tile([128, 512], F32, tag="ps")
                        for f in range(4):
                            nc.tensor.matmul(
                                pw[:], oT_sb[f][:, tsl],
                                wo_sb[f][:, 512 * j:512 * (j + 1)],
                                start=(f == 0), stop=(f == 3),
                            )
                        ost = outst.tile([128, 512], F32, tag="outst")
                        nc.vector.tensor_copy(ost[:], pw[:])
                        nc.sync.dma_start(
                            d_part[tsl, 512 * j:512 * (j + 1)], ost[:]
                        )

    _split_multiwaits(nc)
    return nc


_PROGRAM = None


def _get_program():
    global _PROGRAM
    if _PROGRAM is None:
        _PROGRAM = build_program()
    return _PROGRAM


def make_in_maps(Q, K, V, Wq, bq, Wk, bk, Wv, bv, Wo, bo):
    f32 = lambda x: np.asarray(x, dtype=np.float32)
    Q, K, V = f32(Q), f32(K), f32(V)
    Wq, Wk, Wv, Wo = f32(Wq), f32(Wk), f32(Wv), f32(Wo)
    bq, bk, bv = f32(bq), f32(bk), f32(bv)
    ones = np.ones((1, 128), np.float32)
    in_maps = []
    for c in range(N_CORES):
        b, hh = c // 2, c % 2
        fs = slice(F * hh, F * (hh + 1))
        in_maps.append({
            "qt": np.ascontiguousarray(Q[b].T),
            "kt": np.ascontiguousarray(K[b].T),
            "vt": np.ascontiguousarray(V[b].T),
            "wq": np.ascontiguousarray(Wq[:, fs]),
            "wk": np.ascontiguousarray(Wk[:, fs]),
            "wv": np.ascontiguousarray(Wv[:, fs]),
            "wo": np.ascontiguousarray(Wo[fs, :]),
            "bq": np.ascontiguousarray(bq[fs]),
            "bk": np.ascontiguousarray(bk[fs]),
            "bv": np.ascontiguousarray(bv[fs]),
            "ones": ones,
        })
    return in_maps


def kernel(Q, K, V, Wq, bq, Wk, bk, Wv, bv, Wo, bo, _trace=False, _trace_kwargs=None):
    nc = _get_program()
    in_maps = make_in_maps(Q, K, V, Wq, bq, Wk, bk, Wv, bv, Wo, bo)
    res = run_bass_kernel_spmd(
        nc, in_maps, core_ids=list(range(N_CORES)),
        trace=_trace, **(_trace_kwargs or {}),
    )
    parts = [r["part"] for r in res.results]
    out = np.stack([parts[2 * b] + parts[2 * b + 1] for b in range(B)])
    out += np.asarray(bo, dtype=np.float32)[None, None, :]
    if _trace:
        return out, res
    return out


# revision 6
# speedup vs baseline: 1.0630x; 1.0186x over previous
"""Multi-head attention (B=4, S=2048, D=1024, H=16) on 8 trn2 NeuronCores.

Sharding: (batch, head-half) -> one core each. Core c handles batch c//2 and
heads (c%2)*8 .. (c%2)*8+7 (feature columns (c%2)*512 .. +512 of the QKV
projections, rows of Wo). Each core computes its 8 heads' attention and a
partial output projection; the host sums the two partials per batch and adds
the output bias.

Device layout per core (S=2048 tokens, F=512 local features, hd=64):
  - inputs Q/K/V arrive host-transposed as [1024, 2048] so the d_model
    contraction sits on SBUF partitions,
  - q^T/k^T are produced feature-major ([512, 2048]) via lhsT=W, rhs=X^T,
  - scores are computed transposed (S^T[k, q]) so the P@V matmul can use v
    in natural [token, feature] layout as the stationary operand,
  - softmax: exp on the ACT engine with the 1/8 scale folded in; the
    denominator comes from an all-ones 65th column appended to v; the
    normalization multiplies o'^T rows by a PE-broadcast reciprocal row.
Matmuls run as float32r (FP22 multiplies, fp32 accumulate) except P@V,
which runs bf16 x bf16 on the exp output.
"""
import numpy as np

import concourse.bass as bass
import concourse.tile as tile
from concourse import mybir
from concourse.bass_utils import run_bass_kernel_spmd

F32 = mybir.dt.float32
F32R = mybir.dt.float32r
BF16 = mybir.dt.bfloat16
EXP = mybir.ActivationFunctionType.Exp

B, S, DM, H_TOT = 4, 2048, 1024, 16
F = 512          # features per core (8 heads x 64)
HD = 64          # head dim
NH = 8           # heads per core
NP = 4           # head pairs per core
KT = 16          # k tiles of 128
NQT = 4          # q chunks of 512
SCALE = 0.125    # 1/sqrt(64)
N_CORES = 8

_WAIT_CAP = {"InstEventSemaphore": 2}


def _split_multiwaits(nc):
    """This walrus build accepts 1 sync-wait per instruction (2 on
    EventSemaphore); spread extras over same-engine NOPs placed before."""
    n_spill = 0
    for f in nc.m.functions:
        for bb in f.blocks:
            new = []
            changed = False
            for inst in bb.instructions:
                si = inst.sync_info
                cap = _WAIT_CAP.get(type(inst).__name__, 1)
                if si is not None and len(si.on_wait) > cap:
                    extra = list(si.on_wait[: len(si.on_wait) - cap])
                    del si.on_wait[: len(si.on_wait) - cap]
                    for w in extra:
                        n_spill += 1
                        nop = mybir.InstNoOp(name=f"I-wspill-{n_spill}-{inst.name}")
                        nop.engine = inst.engine
                        nop.sync_info = mybir.SyncInfo(on_wait=[w], on_update=[])
                        new.append(nop)
                    changed = True
                new.append(inst)
            if changed:
                bb.instructions[:] = new
    return n_spill


def build_program():
    nc = bass.Bass("TRN2", target_bir_lowering=False, debug=False, num_devices=1)

    d_qt = nc.dram_tensor("qt", [DM, S], BF16, kind="ExternalInput").ap()
    d_kt = nc.dram_tensor("kt", [DM, S], BF16, kind="ExternalInput").ap()
    d_vt = nc.dram_tensor("vt", [DM, S], BF16, kind="ExternalInput").ap()
    d_wq = nc.dram_tensor("wq", [DM, F], BF16, kind="ExternalInput").ap()
    d_wk = nc.dram_tensor("wk", [DM, F], BF16, kind="ExternalInput").ap()
    d_wv = nc.dram_tensor("wv", [DM, F], BF16, kind="ExternalInput").ap()
    d_wo = nc.dram_tensor("wo", [F, DM], BF16, kind="ExternalInput").ap()
    d_bq = nc.dram_tensor("bq", [F], F32, kind="ExternalInput").ap()
    d_bk = nc.dram_tensor("bk", [F], F32, kind="ExternalInput").ap()
    d_bv = nc.dram_tensor("bv", [F], F32R, kind="ExternalInput").ap()
    d_ones = nc.dram_tensor("ones", [1, 128], F32R, kind="ExternalInput").ap()
    d_part = nc.dram_tensor("part", [S, DM], F32, kind="ExternalOutput").ap()

    with tile.TileContext(nc) as tc:
        with (
            tc.tile_pool(name="wpool", bufs=1) as wpool,
            tc.tile_pool(name="big", bufs=1) as big,
            tc.tile_pool(name="inch", bufs=4) as inch,
            tc.tile_pool(name="wch", bufs=4) as wch,
            tc.tile_pool(name="vtch", bufs=4) as vtch,
            tc.tile_pool(name="exch", bufs=6) as exch,
            tc.tile_pool(name="small", bufs=4) as small,
            tc.tile_pool(name="outst", bufs=2) as outst,
            tc.tile_pool(name="rcp", bufs=2) as rcp,
            tc.tile_pool(name="ps_sc", bufs=2, space="PSUM") as ps_sc,
            tc.tile_pool(name="ps_pv", bufs=4, space="PSUM") as ps_pv,
        ):
            # ---- resident tiles
            wv_sb = [wpool.tile([128, F], BF16, tag=f"wv{m}", name=f"wv{m}") for m in range(8)]
            wo_sb = [wpool.tile([128, DM], BF16, tag=f"wo{f}", name=f"wo{f}") for f in range(4)]
            qT_sb = [big.tile([128, S], BF16, tag=f"qT{f}", name=f"qT{f}") for f in range(4)]
            kT_sb = [big.tile([128, S], BF16, tag=f"kT{f}", name=f"kT{f}") for f in range(4)]
            oT_sb = [big.tile([128, S], BF16, tag=f"oT{f}", name=f"oT{f}") for f in range(4)]
            v_sb = [big.tile([128, NH * (HD + 1)], BF16, tag=f"v{t}", name=f"v{t}") for t in range(KT)]
            bq_sb = wpool.tile([128, 4], F32, tag="bq")
            bk_sb = wpool.tile([128, 4], F32, tag="bk")
            bv_sb = wpool.tile([1, F], F32R, tag="bv")
            ones_sb = wpool.tile([1, 128], F32R, tag="ones")
            bvbc_sb = wpool.tile([128, F], F32, tag="bvbc")

            for m in range(8):
                nc.sync.dma_start(wv_sb[m][:], d_wv[128 * m:128 * (m + 1), :])
            for f in range(4):
                nc.sync.dma_start(wo_sb[f][:], d_wo[128 * f:128 * (f + 1), :])
            nc.sync.dma_start(bq_sb[:], d_bq.rearrange("(f p) -> p f", p=128))
            nc.sync.dma_start(bk_sb[:], d_bk.rearrange("(f p) -> p f", p=128))
            nc.sync.dma_start(bv_sb[:], d_bv.rearrange("(a f) -> a f", a=1))
            nc.sync.dma_start(ones_sb[:], d_ones[:])

            # bv broadcast over partitions via K=1 matmul (biases are usually
            # zero here, but keep the math general)
            psbv = ps_sc.tile([128, 1024], F32, tag="sc", name="psbv")
            nc.tensor.matmul(psbv[:, 0:512], ones_sb[0:1, :], bv_sb[0:1, :])
            nc.vector.tensor_copy(bvbc_sb[:], psbv[:, 0:512])

            # ---- projections producing transposed outputs: k^T then q^T
            def project_T(w_dram, src, bias_sb, dst_sb):
                for n in range(NQT):
                    for fh in range(2):
                        acc = [
                            ps_pv.tile([128, 512], F32, tag="po", name="acc0"),
                            ps_pv.tile([128, 512], F32, tag="po", name="acc1"),
                        ]
                        for m in range(8):
                            wc = wch.tile([128, 256], BF16, tag="wch", name="wc")
                            nc.sync.dma_start(
                                wc[:],
                                w_dram[128 * m:128 * (m + 1),
                                       256 * fh:256 * (fh + 1)],
                            )
                            ch = inch.tile([128, 512], BF16, tag="inch")
                            nc.sync.dma_start(
                                ch[:],
                                src[128 * m:128 * (m + 1), 512 * n:512 * (n + 1)],
                            )
                            for i in range(2):
                                nc.tensor.matmul(
                                    acc[i][:],
                                    wc[:, 128 * i:128 * (i + 1)],
                                    ch[:],
                                    start=(m == 0),
                                    stop=(m == 7),
                                )
                        for i in range(2):
                            f = 2 * fh + i
                            with nc.allow_low_precision(reason="bf16 qT/kT store"):
                                nc.vector.tensor_scalar_add(
                                    dst_sb[f][:, 512 * n:512 * (n + 1)],
                                    acc[i][:],
                                    bias_sb[:, f:f + 1],
                                )

            project_T(d_wk, d_kt, bk_sb, kT_sb)

            # ---- v projection (natural layout, bf16, ones column per head)
            for t in range(KT):
                acc = ps_pv.tile([128, 512], F32, tag="po", name="accv")
                for m in range(8):
                    ch = vtch.tile([128, 128], BF16, tag="vtch")
                    nc.sync.dma_start(
                        ch[:], d_vt[128 * m:128 * (m + 1), 128 * t:128 * (t + 1)]
                    )
                    nc.tensor.matmul(
                        acc[:], ch[:], wv_sb[m][:], start=(m == 0), stop=(m == 7)
                    )
                v3 = v_sb[t][:].rearrange("p (h e) -> p h e", e=HD + 1)
                nc.vector.memset(v3[:, :, HD:HD + 1], 1.0)
                nc.vector.tensor_add(
                    v3[:, :, 0:HD],
                    acc[:].rearrange("p (h e) -> p h e", e=HD),
                    bvbc_sb[:].rearrange("p (h e) -> p h e", e=HD),
                )

            project_T(d_wq, d_qt, bq_sb, qT_sb)

            # ---- attention + output projection, q-chunk major
            wo_pending = []

            def emit_wo(count):
                for _ in range(count):
                    if not wo_pending:
                        return
                    tt, j = wo_pending.pop(0)
                    tsl = slice(128 * tt, 128 * (tt + 1))
                    pw = ps_sc.tile([128, 1024], F32, tag="sc", name="pw")
                    for f in range(4):
                        nc.tensor.matmul(
                            pw[:, 0:512], oT_sb[f][:, tsl],
                            wo_sb[f][:, 512 * j:512 * (j + 1)],
                            start=(f == 0), stop=(f == 3),
                        )
                    ost = outst.tile([128, 512], F32, tag="outst")
                    nc.vector.tensor_copy(ost[:], pw[:, 0:512])
                    nc.sync.dma_start(
                        d_part[tsl, 512 * j:512 * (j + 1)], ost[:]
                    )

            for n in range(NQT):
                qsl = slice(512 * n, 512 * (n + 1))
                for p in range(NP):
                    emit_wo(2)
                    poA = ps_pv.tile([128, 512], F32, tag="po")
                    poB = ps_pv.tile([128, 512], F32, tag="po")

                    def sc_emit(m, p=p, qsl=qsl):
                        scp = ps_sc.tile([128, 1024], F32, tag="sc")
                        ksl = slice(128 * m, 128 * (m + 1))
                        nc.tensor.matmul(
                            scp[:, 0:512], kT_sb[p][0:64, ksl], qT_sb[p][0:64, qsl],
                            tile_position=(0, 0),
                        )
                        nc.tensor.matmul(
                            scp[:, 512:1024], kT_sb[p][64:128, ksl],
                            qT_sb[p][64:128, qsl], tile_position=(64, 0),
                        )
                        ex = exch.tile([128, 1024], BF16, tag="ex")
                        nc.scalar.activation(ex[:], scp[:], EXP, scale=SCALE)
                        return ex

                    exs = {0: sc_emit(0), 1: sc_emit(1)}
                    for m in range(KT):
                        if m + 2 < KT:
                            exs[m + 2] = sc_emit(m + 2)
                        ex = exs.pop(m)
                        nc.tensor.matmul(
                            poA[0:65, :], v_sb[m][:, 130 * p:130 * p + 65],
                            ex[:, 0:512], start=(m == 0), stop=(m == KT - 1),
                        )
                        nc.tensor.matmul(
                            poB[0:65, :], v_sb[m][:, 130 * p + 65:130 * p + 130],
                            ex[:, 512:1024], start=(m == 0), stop=(m == KT - 1),
                        )
                    for i, po in ((0, poA), (1, poB)):
                        r0 = 64 * i
                        dn = small.tile([1, 512], F32R, tag="dn", name="dn")
                        nc.vector.tensor_copy(dn[0:1, :], po[64:65, :])
                        dnr = small.tile([1, 512], F32R, tag="dn", name="dnr")
                        with nc.allow_low_precision(reason="f32r reciprocal"):
                            nc.vector.reciprocal(dnr[0:1, :], dn[0:1, :])
                        pb = ps_sc.tile([128, 1024], F32, tag="sc", name="pb")
                        nc.tensor.matmul(pb[0:64, 0:512], ones_sb[0:1, 0:64], dnr[0:1, :])
                        rc = rcp.tile([64, 512], F32, tag="rc", name="rc")
                        nc.vector.tensor_copy(rc[:], pb[0:64, 0:512])
                        with nc.allow_low_precision(reason="f32r normalized out"):
                            nc.vector.tensor_mul(
                                oT_sb[p][r0:r0 + 64, qsl], po[0:64, :], rc[:]
                            )
                # queue this chunk's Wo pieces (interleaved into later groups)
                for t in range(4):
                    for j in range(2):
                        wo_pending.append((4 * n + t, j))
            emit_wo(len(wo_pending))

    _split_multiwaits(nc)
    return nc


_PROGRAM = None


def _get_program():
    global _PROGRAM
    if _PROGRAM is None:
        _PROGRAM = build_program()
    return _PROGRAM


def make_in_maps(Q, K, V, Wq, bq, Wk, bk, Wv, bv, Wo, bo):
    import ml_dtypes
    bf = lambda x: np.asarray(x, dtype=np.float32).astype(ml_dtypes.bfloat16)
    f32 = lambda x: np.asarray(x, dtype=np.float32)
    Q, K, V = bf(Q), bf(K), bf(V)
    Wq, Wk, Wv, Wo = bf(Wq), bf(Wk), bf(Wv), bf(Wo)
    bq, bk, bv = f32(bq), f32(bk), f32(bv)
    ones = np.ones((1, 128), np.float32)
    in_maps = []
    for c in range(N_CORES):
        b, hh = c // 2, c % 2
        fs = slice(F * hh, F * (hh + 1))
        in_maps.append({
            "qt": np.ascontiguousarray(Q[b].T),
            "kt": np.ascontiguousarray(K[b].T),
            "vt": np.ascontiguousarray(V[b].T),
            "wq": np.ascontiguousarray(Wq[:, fs]),
            "wk": np.ascontiguousarray(Wk[:, fs]),
            "wv": np.ascontiguousarray(Wv[:, fs]),
            "wo": np.ascontiguousarray(Wo[fs, :]),
            "bq": np.ascontiguousarray(bq[fs]),
            "bk": np.ascontiguousarray(bk[fs]),
            "bv": np.ascontiguousarray(bv[fs]),
            "ones": ones,
        })
    return in_maps


def kernel(Q, K, V, Wq, bq, Wk, bk, Wv, bv, Wo, bo, _trace=False, _trace_kwargs=None):
    nc = _get_program()
    in_maps = make_in_maps(Q, K, V, Wq, bq, Wk, bk, Wv, bv, Wo, bo)
    res = run_bass_kernel_spmd(
        nc, in_maps, core_ids=list(range(N_CORES)),
        trace=_trace, **(_trace_kwargs or {}),
    )
    parts = [r["part"] for r in res.results]
    out = np.stack([parts[2 * b] + parts[2 * b + 1] for b in range(B)])
    out += np.asarray(bo, dtype=np.float32)[None, None, :]
    if _trace:
        return out, res
    return out
